# revision 8
# baseline (speedup 1.0000x reference)
import numpy as np
from contextlib import ExitStack

import concourse.bass as bass
import concourse.bacc as bacc
import concourse.mybir as mybir
from concourse import tile
from concourse.masks import make_identity

BF16 = mybir.dt.bfloat16
F32 = mybir.dt.float32
AF = mybir.ActivationFunctionType

D_MODEL = 768
N_HEADS = 12
HD = 64
N_CORES = 8
NH_LOC = 3
DC = D_MODEL // 128
CHUNK = 512
GRP = 3


def build(nc, S, level=3):
    SB = S // 128
    NCH = S // CHUNK
    KB = S // 128

    xT_d = nc.declare_dram_parameter("xT", [DC, 128, S], BF16, isOutput=False)
    wqk_d = nc.declare_dram_parameter("wqk", [4, DC, 128, 128], BF16, isOutput=False)
    bqk_d = nc.declare_dram_parameter("bqk", [128, 4], F32, isOutput=False)
    wv_d = nc.declare_dram_parameter("wv", [DC, 128, NH_LOC * HD], BF16, isOutput=False)
    wo_d = nc.declare_dram_parameter("wo", [3, HD, D_MODEL], BF16, isOutput=False)
    out_d = nc.declare_dram_parameter("out", [S, D_MODEL], F32, isOutput=True)

    HEAD_SLOT = [(0, 0), (0, 64), (1, 0)]

    with tile.TileContext(nc) as tc, ExitStack() as ctx:
        const = ctx.enter_context(tc.tile_pool(name="const", bufs=1))

        def ctile(name, shape, dt):
            return const.tile(shape, dt, tag=name, name=name)

        xts = [ctile(f"xt{i}", [128, S], BF16) for i in range(DC)]
        wqks = [ctile(f"wqk{i}", [128, DC * 128], BF16) for i in range(4)]
        bqks = ctile("bqk", [128, 4], F32)
        wvs = [ctile(f"wv{i}", [128, NH_LOC * HD], BF16) for i in range(DC)]
        wos = [ctile(f"wo{i}", [HD, D_MODEL], BF16) for i in range(NH_LOC)]
        ident = ctile("ident", [128, 128], BF16)
        v1s = [ctile(f"v1_{h}", [128, 65 * KB], BF16) for h in range(NH_LOC)]
        qks = [ctile(f"qk{i}", [128, S], BF16) for i in range(4)]
        a_sb = [ctile(f"a{h}", [128, SB * HD], BF16) for h in range(NH_LOC)]
        ats = [ctile(f"at{i}", [HD, S], BF16) for i in range(NH_LOC)]

        pt_pool = ctx.enter_context(tc.tile_pool(name="pt", bufs=12))
        outst_pool = ctx.enter_context(tc.tile_pool(name="outst", bufs=3))
        small_pool = ctx.enter_context(tc.tile_pool(name="small", bufs=6))

        for i in range(DC):
            nc.sync.dma_start(xts[i][:], xT_d[i])
        for blk in range(4):
            for dcc in range(DC):
                nc.sync.dma_start(
                    wqks[blk][:, dcc * 128:(dcc + 1) * 128], wqk_d[blk, dcc]
                )
        nc.sync.dma_start(bqks[:], bqk_d[:])
        for i in range(DC):
            nc.sync.dma_start(wvs[i][:], wv_d[i])
        for i in range(NH_LOC):
            nc.sync.dma_start(wos[i][:], wo_d[i])
        make_identity(nc, ident[:])
        for h in range(NH_LOC):
            nc.gpsimd.memset(v1s[h][:], 1.0)

        with tc.tile_pool(name="ps_proj", bufs=3, space="PSUM") as ps1:
            for blk in range(4):
                for sc in range(S // 512):
                    pp = ps1.tile([128, 512], F32, tag="pp", name=f"pp{blk}_{sc}")
                    for dcc in range(DC):
                        nc.tensor.matmul(
                            pp[:],
                            lhsT=wqks[blk][:, dcc * 128:(dcc + 1) * 128],
                            rhs=xts[dcc][:, sc * 512:(sc + 1) * 512],
                            start=(dcc == 0),
                            stop=(dcc == DC - 1),
                        )
                    nc.vector.tensor_scalar_add(
                        qks[blk][:, sc * 512:(sc + 1) * 512],
                        pp[:],
                        bqks[:, blk:blk + 1],
                    )
            for sb in range(SB):
                pv = ps1.tile([128, 512], F32, tag="pp", name=f"pv{sb}")
                pvv = pv[:, 0:NH_LOC * HD]
                for dcc in range(DC):
                    nc.tensor.matmul(
                        pvv,
                        lhsT=xts[dcc][:, sb * 128:(sb + 1) * 128],
                        rhs=wvs[dcc][:],
                        start=(dcc == 0),
                        stop=(dcc == DC - 1),
                    )
                for h in range(NH_LOC):
                    nc.vector.tensor_copy(
                        v1s[h][:, sb * 65: sb * 65 + 64],
                        pv[:, h * HD:(h + 1) * HD],
                    )

        if level < 2:
            for sb in range(SB):
                ost = outst_pool.tile([128, D_MODEL], F32, tag="ost",
                                      name=f"ost{sb}")
                nc.vector.memset(ost[:], 0.0)
                nc.sync.dma_start(out_d[sb * 128:(sb + 1) * 128, :], ost[:])
            return nc

        groups = []
        j0 = 0
        while j0 < KB:
            groups.append((j0, min(GRP, KB - j0)))
            j0 += GRP

        with (
            tc.tile_pool(name="ps_st", bufs=2, space="PSUM") as ps_st,
            tc.tile_pool(name="ps_acc", bufs=2, space="PSUM") as ps_acc,
        ):
            for h in range(NH_LOC):
                qt_i, off = HEAD_SLOT[h]
                qt = qks[2 * qt_i]
                kt = qks[2 * qt_i + 1]
                for qc in range(NCH):
                    pts = []
                    for (g0, glen) in groups:
                        st = ps_st.tile([128, GRP * CHUNK], F32, tag="st",
                                        name=f"st{h}_{qc}_{g0}")
                        for t in range(glen):
                            j = g0 + t
                            nc.tensor.matmul(
                                st[:, t * CHUNK:(t + 1) * CHUNK],
                                lhsT=kt[off:off + HD, j * 128:(j + 1) * 128],
                                rhs=qt[off:off + HD, qc * CHUNK:(qc + 1) * CHUNK],
                                start=True,
                                stop=True,
                            )
                        pt = pt_pool.tile([128, GRP * CHUNK], BF16, tag="pt",
                                          name=f"pt{h}_{qc}_{g0}")
                        nc.scalar.activation(
                            pt[:, 0:glen * CHUNK],
                            st[:, 0:glen * CHUNK],
                            AF.Exp,
                            scale=0.125,
                        )
                        pts.append((g0, pt))
                    for i in range(CHUNK // 128):
                        acc = ps_acc.tile([128, 65], F32, tag="acc",
                                          name=f"acc{h}_{qc}_{i}")
                        for (g0, pt) in pts:
                            for t in range(GRP):
                                j = g0 + t
                                if j >= KB:
                                    break
                                nc.tensor.matmul(
                                    acc[:],
                                    lhsT=pt[:, t * CHUNK + i * 128:
                                            t * CHUNK + (i + 1) * 128],
                                    rhs=v1s[h][:, j * 65:(j + 1) * 65],
                                    start=(j == 0),
                                    stop=(j == KB - 1),
                                )
                        qb = qc * (CHUNK // 128) + i
                        rec = small_pool.tile([128, 1], F32, tag="rec",
                                              name=f"rec{h}_{qb}")
                        nc.vector.reciprocal(rec[:], acc[:, 64:65])
                        nc.vector.tensor_scalar_mul(
                            a_sb[h][:, qb * HD:(qb + 1) * HD],
                            acc[:, 0:HD],
                            rec[:],
                        )

        if level < 3:
            for sb in range(SB):
                ost = outst_pool.tile([128, D_MODEL], F32, tag="ost",
                                      name=f"ost{sb}")
                nc.vector.memset(ost[:], 0.0)
                nc.sync.dma_start(out_d[sb * 128:(sb + 1) * 128, :], ost[:])
            return nc

        with tc.tile_pool(name="ps_fin", bufs=2, space="PSUM") as ps_fin:
            for h in range(NH_LOC):
                for sb in range(SB):
                    pst = ps_fin.tile([128, 128], BF16, tag="tp",
                                      name=f"tp{h}_{sb}")
                    nc.tensor.transpose(
                        pst[0:HD, :],
                        a_sb[h][:, sb * HD:(sb + 1) * HD],
                        ident[:],
                    )
                    nc.vector.tensor_copy(
                        ats[h][:, sb * 128:(sb + 1) * 128],
                        pst[0:HD, :],
                    )
            for sb in range(SB):
                ost = outst_pool.tile([128, D_MODEL], F32, tag="ost",
                                      name=f"ost{sb}")
                for (n0, n1, tag) in ((0, 512, "fp1"), (512, D_MODEL, "fp2")):
                    po = ps_fin.tile([128, n1 - n0], F32, tag=tag,
                                     name=f"{tag}_{sb}")
                    for h in range(NH_LOC):
                        nc.tensor.matmul(
                            po[:],
                            lhsT=ats[h][:, sb * 128:(sb + 1) * 128],
                            rhs=wos[h][:, n0:n1],
                            start=(h == 0),
                            stop=(h == NH_LOC - 1),
                        )
                    nc.vector.tensor_copy(ost[:, n0:n1], po[:])
                nc.sync.dma_start(out_d[sb * 128:(sb + 1) * 128, :], ost[:])

    return nc


def make_nc(S=4096, level=3):
    nc = bacc.Bacc(None, target_bir_lowering=False, debug=False)
    build(nc, S, level=level)
    nc.compile()
    return nc


def shard_inputs(x, Wq, bq, Wk, bk, Wv, bv, Wo, bo, S):
    import ml_dtypes

    bf = ml_dtypes.bfloat16
    in_maps = []
    for c in range(N_CORES):
        b = c // 4
        h0 = NH_LOC * (c % 4)
        cs, ce = h0 * HD, (h0 + NH_LOC) * HD
        xT = np.ascontiguousarray(x[b].T).astype(bf).reshape(DC, 128, S)

        def blkify(w2):
            return np.ascontiguousarray(w2).astype(bf).reshape(DC, 128, 128)

        pad = np.zeros((D_MODEL, HD), np.float32)
        wqk = np.stack([
            blkify(Wq[:, cs:cs + 2 * HD]),
            blkify(Wk[:, cs:cs + 2 * HD]),
            blkify(np.concatenate([Wq[:, cs + 2 * HD:ce], pad], axis=1)),
            blkify(np.concatenate([Wk[:, cs + 2 * HD:ce], pad], axis=1)),
        ])
        zpad = np.zeros(HD, np.float32)
        bqk = np.stack([
            bq[cs:cs + 2 * HD],
            bk[cs:cs + 2 * HD],
            np.concatenate([bq[cs + 2 * HD:ce], zpad]),
            np.concatenate([bk[cs + 2 * HD:ce], zpad]),
        ], axis=1).astype(np.float32)
        wv = np.ascontiguousarray(Wv[:, cs:ce]).astype(bf).reshape(
            DC, 128, NH_LOC * HD)
        wo = np.ascontiguousarray(Wo[cs:ce, :]).astype(bf).reshape(
            NH_LOC, HD, D_MODEL)
        in_maps.append({"xT": xT, "wqk": wqk, "bqk": bqk, "wv": wv, "wo": wo})
    return in_maps


_NC_CACHE = {}


def kernel(x, Wq, bq, Wk, bk, Wv, bv, Wo, bo):
    from concourse import bass_utils

    x = np.asarray(x)
    B, S, D = x.shape
    assert (B, D) == (2, D_MODEL)
    if S not in _NC_CACHE:
        _NC_CACHE[S] = make_nc(S)
    nc = _NC_CACHE[S]

    in_maps = shard_inputs(x, Wq, bq, Wk, bk, Wv, bv, Wo, bo, S)
    res = bass_utils.run_bass_kernel_spmd(nc, in_maps, core_ids=list(range(N_CORES)))

    bias = (bo.astype(np.float32)
            + bv.astype(np.float32) @ Wo.astype(np.float32))
    out = np.empty((B, S, D_MODEL), np.float32)
    for b in range(B):
        acc = res.results[4 * b]["out"].astype(np.float32).copy()
        for c in range(4 * b + 1, 4 * b + 4):
            acc += res.results[c]["out"]
        out[b] = acc + bias
    return out
